# revision 6
# baseline (speedup 1.0000x reference)
"""CascadeNNBN Trainium2 kernel.

8-way data-parallel over the batch dim. Each core holds a 2048-row shard
of the batch with features kept TRANSPOSED in SBUF (features on
partitions, batch on the free axis), so every cascade matmul contracts
over the partition dim with no on-device transposes:

    h_i^T [256, 2048] = W_i @ feats^T   (lhsT = W_i^T, host-pretransposed)

BatchNorm batch statistics are produced per-core with bn_stats/bn_aggr,
converted to (mean-sum, E[h^2]-sum) pairs and AllReduce-added across the
8 cores (8 tiny 2KB collectives, one per stage). The per-stage collective
latency is hidden behind the next stage's matmuls on the feature blocks
that do not depend on the not-yet-normalized output (BN is a per-feature
affine transform, so only the last two k-tiles of each stage's
contraction are gated on the AllReduce result).

Matmuls run in bf16 (fp32 PSUM accumulation); statistics, normalization
coefficients and the final output are fp32.
"""

import sys

if "/opt/trn_rl_repo" not in sys.path:
    sys.path.insert(0, "/opt/trn_rl_repo")

import numpy as np
from ml_dtypes import bfloat16

import concourse.bass as bass  # noqa: F401  (import keeps bass registered)
import concourse.mybir as mybir
import concourse.tile as tile
from concourse import bacc
from concourse.bass_utils import run_bass_kernel_spmd

N_CORES = 8
B = 16384
BSH = B // N_CORES          # 2048 batch rows per core
DIN = 512
K = 8                       # cascade stages
WS = 256                    # neurons per stage
DOUT = 128
EPS = 1e-5
P = 128
NB = BSH // 512             # batch chunks of 512 (PSUM bank free dim)
KO = [(DIN + WS * i) // P for i in range(K)]   # k-tiles per stage: 4,6,...,18
T_TOT = (DIN + WS * K) // P                    # 20 F tiles

_NC_CACHE = {}

# test-harness knobs (ignored in normal use): when TRACE_DIR is set the
# device run is profiled and kernel() stores the BassKernelResults here.
TRACE_DIR = None
LAST_RESULTS = None

BF = mybir.dt.bfloat16
F32 = mybir.dt.float32


def _build_nc():
    nc = bacc.Bacc("TRN2", target_bir_lowering=False, debug=False,
                   num_devices=N_CORES)

    xt_d = nc.dram_tensor("xt", [P, DIN // P, BSH], BF, kind="ExternalInput")
    w_d = [
        nc.dram_tensor(f"w{i}", [P, KO[i], WS], BF, kind="ExternalInput")
        for i in range(K)
    ]
    wo_d = nc.dram_tensor("wo", [P, T_TOT, DOUT], BF, kind="ExternalInput")
    bv_d = nc.dram_tensor("bv", [P, K, 2], F32, kind="ExternalInput")
    gv_d = nc.dram_tensor("gv", [P, K, 2], F32, kind="ExternalInput")
    bev_d = nc.dram_tensor("bev", [P, K, 2], F32, kind="ExternalInput")
    bout_d = nc.dram_tensor("boutv", [P, 1], F32, kind="ExternalInput")
    outT_d = nc.dram_tensor("outT", [P, BSH], F32, kind="ExternalOutput")

    with tile.TileContext(nc) as tc:
        _emit(nc, tc, xt_d, w_d, wo_d, bv_d, gv_d, bev_d, bout_d, outT_d)
    nc.compile()
    return nc


def _emit(nc, tc, xt_d, w_d, wo_d, bv_d, gv_d, bev_d, bout_d, outT_d):
    AF = mybir.ActivationFunctionType
    OP = mybir.AluOpType
    groups = [list(range(N_CORES))]

    with (
        tc.tile_pool(name="big", bufs=1) as big,
        tc.tile_pool(name="hp", bufs=2) as hp,
        tc.tile_pool(name="small", bufs=2) as small,
        tc.tile_pool(name="ps", bufs=8, space="PSUM") as ps,
        tc.tile_pool(name="dram", bufs=2, space="DRAM") as dram,
    ):
        # ---- persistent SBUF ----
        F = [big.tile([P, BSH], BF, tag=f"F{t}", name=f"F{t}") for t in range(T_TOT)]
        Wsb = [big.tile([P, KO[i], WS], BF, tag=f"W{i}", name=f"W{i}") for i in range(K)]
        WO = big.tile([P, T_TOT, DOUT], BF, tag="WO")
        BV = big.tile([P, K, 2], F32, tag="BV")
        GV = big.tile([P, K, 2], F32, tag="GV")
        BEV = big.tile([P, K, 2], F32, tag="BEV")
        BOUT = big.tile([P, 1], F32, tag="BOUT")
        OUTSB = big.tile([P, BSH], F32, tag="OUTSB")
        EPSC = big.tile([P, 1], F32, tag="EPSC")
        nc.vector.memset(EPSC[:], EPS)

        # ---- input DMAs ----
        for t in range(DIN // P):
            nc.sync.dma_start(F[t][:, :], xt_d[:, t, :])
        for i in range(K):
            nc.sync.dma_start(Wsb[i][:], w_d[i][:, :, :])
        nc.sync.dma_start(WO[:], wo_d[:, :, :])
        nc.sync.dma_start(BV[:], bv_d[:, :, :])
        nc.sync.dma_start(GV[:], gv_d[:, :, :])
        nc.sync.dma_start(BEV[:], bev_d[:, :, :])
        nc.sync.dma_start(BOUT[:], bout_d[:, :])

        def stage_mms(i, psums, ks):
            last = KO[i] - 1
            for n in range(2):
                for k in ks:
                    lhsT = Wsb[i][:, k, n * P:(n + 1) * P]
                    for bb in range(NB):
                        nc.tensor.matmul(
                            psums[n][bb][:, :],
                            lhsT,
                            F[k][:, bb * 512:(bb + 1) * 512],
                            start=(k == 0),
                            stop=(k == last),
                        )

        def out_mms(pso, ks):
            for k in ks:
                lhsT = WO[:, k, :]
                for bb in range(NB):
                    nc.tensor.matmul(
                        pso[bb][:, :],
                        lhsT,
                        F[k][:, bb * 512:(bb + 1) * 512],
                        start=(k == 0),
                        stop=(k == T_TOT - 1),
                    )

        def alloc_stage_psums():
            return [
                [ps.tile([P, 512], F32, tag="pt", name="pt") for _ in range(NB)]
                for _ in range(2)
            ]

        # stage 0: everything available immediately
        psums = alloc_stage_psums()
        stage_mms(0, psums, range(KO[0]))

        for i in range(K):
            # ---- relu + bias: PSUM -> fp32 h in SBUF (ScalarE) ----
            hs = [hp.tile([P, BSH], F32, tag=f"h{n}", name=f"h{n}") for n in range(2)]
            for n in range(2):
                for bb in range(NB):
                    nc.scalar.activation(
                        hs[n][:, bb * 512:(bb + 1) * 512],
                        psums[n][bb][:, :],
                        AF.Relu,
                        bias=BV[:, i, n:n + 1],
                        scale=1.0,
                    )

            # ---- per-core batch stats (VectorE) ----
            st = small.tile([P, 2, NB, 6], F32, tag="st")
            mv = small.tile([P, 2, 2], F32, tag="mv")
            arin = small.tile([P, 2, 2], F32, tag="arin")
            for n in range(2):
                hv = hs[n][:, :].rearrange("p (c f) -> p c f", f=512)
                for c in range(NB):
                    nc.vector.bn_stats(st[:, n, c, :], hv[:, c, :])
                nc.vector.bn_aggr(mv[:, n, :], st[:, n, :, :])
                # arin[:,n,0] = per-core mean; arin[:,n,1] = per-core E[h^2]
                nc.gpsimd.tensor_copy(arin[:, n, 0:1], mv[:, n, 0:1])
                nc.vector.tensor_scalar(
                    arin[:, n, 1:2], mv[:, n, 0:1], mv[:, n, 0:1],
                    mv[:, n, 1:2], op0=OP.mult, op1=OP.add,
                )

            # ---- cross-core AllReduce of the stats (2KB) ----
            ccin = dram.tile([P, 2, 2], F32, tag="ccin")
            ccout = dram.tile([P, 2, 2], F32, tag="ccout", addr_space="Shared")
            nc.sync.dma_start(ccin[:], arin[:])
            nc.gpsimd.collective_compute(
                "AllReduce", OP.add, replica_groups=groups,
                ins=[ccin.opt()], outs=[ccout.opt()],
            )
            ared = small.tile([P, 2, 2], F32, tag="ared")
            nc.sync.dma_start(ared[:], ccout[:])

            # ---- overlap window: next stage's AR-independent matmuls ----
            if i < K - 1:
                psums = alloc_stage_psums()
                stage_mms(i + 1, psums, range(KO[i + 1] - 2))
            else:
                pso = [ps.tile([P, 512], F32, tag="pt", name="pt") for _ in range(NB)]
                out_mms(pso, range(T_TOT - 2))

            # ---- BN affine coefficients from global stats ----
            mu = small.tile([P, 2], F32, tag="mu")
            e2 = small.tile([P, 2], F32, tag="e2")
            nv = small.tile([P, 2], F32, tag="nv")
            rstd = small.tile([P, 2], F32, tag="rstd")
            a_ = small.tile([P, 2], F32, tag="a_")
            negc = small.tile([P, 2], F32, tag="negc")
            nc.scalar.mul(mu[:], ared[:, :, 0], 1.0 / N_CORES)
            nc.scalar.mul(e2[:], ared[:, :, 1], 1.0 / N_CORES)
            nc.vector.tensor_mul(nv[:], mu[:], mu[:])
            nc.vector.tensor_sub(nv[:], nv[:], e2[:])          # mu^2 - E2 = -var
            nc.scalar.activation(rstd[:], nv[:], AF.Sqrt,
                                 bias=EPSC[:, 0:1], scale=-1.0)  # sqrt(var+eps)
            nc.vector.reciprocal(rstd[:], rstd[:])
            nc.vector.tensor_mul(a_[:], GV[:, i, :], rstd[:])
            nc.vector.tensor_mul(negc[:], mu[:], a_[:])
            nc.vector.tensor_sub(negc[:], negc[:], BEV[:, i, :])  # a*mu - beta

            # ---- normalize into the F blocks (bf16) ----
            for n in range(2):
                nc.vector.tensor_scalar(
                    F[DIN // P + 2 * i + n][:, :], hs[n][:, :],
                    a_[:, n:n + 1], negc[:, n:n + 1],
                    op0=OP.mult, op1=OP.subtract,
                )

            # ---- gated (late) matmuls of the next stage ----
            if i < K - 1:
                stage_mms(i + 1, psums, [KO[i + 1] - 2, KO[i + 1] - 1])
            else:
                out_mms(pso, [T_TOT - 2, T_TOT - 1])

        # ---- epilogue: bias add + store ----
        for bb in range(NB):
            nc.vector.tensor_scalar_add(
                OUTSB[:, bb * 512:(bb + 1) * 512], pso[bb][:, :], BOUT[:, 0:1],
            )
        nc.sync.dma_start(outT_d[:, :], OUTSB[:])


def _get_nc():
    if "nc" not in _NC_CACHE:
        _NC_CACHE["nc"] = _build_nc()
    return _NC_CACHE["nc"]


def kernel(x, W0, W1, W2, W3, W4, W5, W6, W7, b, gamma, beta, Wout, bout):
    Ws = [W0, W1, W2, W3, W4, W5, W6, W7]
    nc = _get_nc()

    def pack_vec(v):  # [8,256] -> [128, 8, 2]
        return np.ascontiguousarray(
            np.asarray(v, np.float32).reshape(K, 2, P).transpose(2, 0, 1))

    common = {}
    for i, W in enumerate(Ws):
        wt = np.asarray(W, np.float32).T.astype(bfloat16)        # [d_i, 256]
        common[f"w{i}"] = np.ascontiguousarray(
            wt.reshape(KO[i], P, WS).transpose(1, 0, 2))         # [128, ko, 256]
    wot = np.asarray(Wout, np.float32).T.astype(bfloat16)        # [2560, 128]
    common["wo"] = np.ascontiguousarray(
        wot.reshape(T_TOT, P, DOUT).transpose(1, 0, 2))          # [128, 20, 128]
    common["bv"] = pack_vec(b)
    common["gv"] = pack_vec(gamma)
    common["bev"] = pack_vec(beta)
    common["boutv"] = np.ascontiguousarray(
        np.asarray(bout, np.float32).reshape(P, 1))

    in_maps = []
    for c in range(N_CORES):
        xs = np.asarray(x[c * BSH:(c + 1) * BSH], np.float32)    # [2048, 512]
        xt = xs.T.astype(bfloat16)                               # [512, 2048]
        in_maps.append({
            **common,
            "xt": np.ascontiguousarray(
                xt.reshape(DIN // P, P, BSH).transpose(1, 0, 2)),
        })

    kw = {}
    if TRACE_DIR is not None:
        kw = dict(trace=True, tmpdir=TRACE_DIR)
    res = run_bass_kernel_spmd(nc, in_maps, list(range(N_CORES)), **kw)
    global LAST_RESULTS
    LAST_RESULTS = res
    out = np.empty((B, DOUT), np.float32)
    for c in range(N_CORES):
        out[c * BSH:(c + 1) * BSH] = res.results[c]["outT"].T
    return out
